# revision 47
# baseline (speedup 1.0000x reference)
"""Trainium2 Bass kernel for nn_CombinedLoss (sinkhorn-KD + soft-CE + embed MSE).

v4 architecture (8 cores, q-sharded logits):
  - batch window FIRST (delta ready before sweeps -> pc gathers run inline
    in every sweep; no held student tiles).
  - One interleaved bf16 tile [B,w,2,QS] per chunk -> ONE XBAR transpose
    (alternating sync/scalar HWDGE queues) -> 2 matmuls per t at N=256
    (halves LDWEIGHTS pressure).
  - bf16 AllReduce (embed packed as hi/lo bf16 pair for f32-exact sum).
  - Joint sinkhorn with 3 wide exps + fast 2-op ln approximation.

  - Per-pair gram sweeps: f32 loads -> bf16 convert (scalar) -> XBAR
    dma-transpose (split across sync/scalar HWDGE queues) -> 4x N=128 bf16
    matmuls per t-step.  No PE transposes, no PSUM staging copies.
  - Sweep order: x0y0, x1y1, batch, x2y2, embeds.  pc gathers for pairs 0/1
    run in the batch window against held bf16 student tiles; pair 2 inline.
  - ONE AllReduce [B,1856] at the end (collectives block DMA progress, so
    they cannot be overlapped with the load stream).
  - Joint 3-pair sinkhorn (9 eps steps) on [B,12] state: one Fh transpose,
    12 PE selector-broadcast matmuls into a 3-bank PSUM tile, row-max
    subtract (pre-scale, for fp32 safety), 12 biased exps with fused
    accumulate, DVE ln-poly (no activation-table thrash).
  - CE + final combine; only core 0's output is consumed.
"""
import os
import numpy as np

B = 128
T = 50
Q = 1024
S = 49          # MAX_STEP - 1
H = 256
NCORES = 8
QS = Q // NCORES
TEMP = 0.5
GSCALE = 1.0 / (TEMP * TEMP)   # p-space gram = GSCALE * logit gram
RHO = 500.0 ** 2
EPS_FINAL = 0.005 ** 2
SUP_W, DIST_W, EMBED_W, LOSS_WEIGHT = 1.0, 0.01, 1.0, 1.0

# embed t-shard split (padded to 7 per core)
ESPLIT = [7, 7, 6, 6, 6, 6, 6, 6]
EOFF = [0, 7, 14, 20, 26, 32, 38, 44]
EPAD = 7

CH = [(0, 25), (25, 25)]
CHB = [(0, 10), (10, 10), (20, 10), (30, 10), (40, 9)]

# arbuf layout: grams p at 512p; pc_p at 1536+64p; a at 1728; embed at 1792
GOFF = [0, 512, 1024]
PCOFF = [1536, 1600, 1664]
AOFF = 1728
EMOFF = 1792          # hi at 1792, lo at 1793
ARF = 1856


def _eps_schedule():
    eps_list = []
    e = 1.0
    while e > EPS_FINAL:
        eps_list.append(e)
        e = e * 0.25
    eps_list.append(EPS_FINAL)
    return eps_list


def build_bass():
    import concourse.bass as bass
    import concourse.bacc as bacc
    import concourse.tile as tile
    from concourse import mybir
    from concourse.masks import make_identity

    f32 = mybir.dt.float32
    f32r = mybir.dt.float32r
    bf16 = mybir.dt.bfloat16
    i32 = mybir.dt.int32
    Alu = mybir.AluOpType
    Act = mybir.ActivationFunctionType
    X = mybir.AxisListType.X

    nc = bacc.Bacc(
        "TRN2",
        target_bir_lowering=False,
        debug=False,
        num_devices=NCORES,
    )

    xs = [nc.declare_dram_parameter(n, [B, T, QS], f32, isOutput=False)
          for n in ("xc", "xt", "xe")]
    ys = [nc.declare_dram_parameter(n, [B, T, QS], f32, isOutput=False)
          for n in ("yc", "yt", "ye")]
    dbc = nc.declare_dram_parameter("dbc", [B, S, QS], f32, isOutput=False)
    dbn = nc.declare_dram_parameter("dbn", [B, S, QS], f32, isOutput=False)
    ehs = nc.declare_dram_parameter("ehs", [B, EPAD, H], f32, isOutput=False)
    eht = nc.declare_dram_parameter("eht", [B, EPAD, H], f32, isOutput=False)
    eds = nc.declare_dram_parameter("eds", [B, EPAD, H], f32, isOutput=False)
    edt = nc.declare_dram_parameter("edt", [B, EPAD, H], f32, isOutput=False)
    emask = nc.declare_dram_parameter("emask", [B, 8], f32, isOutput=False)
    pmask = nc.declare_dram_parameter("pmask", [B, 4], f32, isOutput=False)
    out_ext = nc.declare_dram_parameter("out", [1, 1], f32, isOutput=True)

    ar_in = nc.dram_tensor("ar_in", [B, EMOFF], bf16)
    ar_out = nc.dram_tensor("ar_out", [B, EMOFF], bf16, addr_space="Shared")
    ar2_in = nc.dram_tensor("ar2_in", [B, 32], bf16)
    ar2_out = nc.dram_tensor("ar2_out", [B, 32], bf16, addr_space="Shared")

    ckd = float(LOSS_WEIGHT * DIST_W * (RHO + EPS_FINAL / 2.0) / B)
    coeff_np = np.full((12, 1), -ckd, np.float32)
    coeff_np[0::4, 0] = ckd   # f_aa
    coeff_np[3::4, 0] = ckd   # g_bb
    coeff_dram = nc.inline_tensor(coeff_np, "coeffc")
    idx_np = np.broadcast_to(np.arange(64, dtype=np.float32), (B, 64)).copy()
    idx_dram = nc.inline_tensor(idx_np, "idxc")
    sel_np = np.zeros((12, 1536), np.float32)
    for k in range(12):
        sel_np[k, 128 * k:128 * (k + 1)] = 1.0
    sel_dram = nc.inline_tensor(sel_np, "selc")

    blog = float(-np.log(float(B)))
    eps_list = _eps_schedule()

    with tile.TileContext(nc) as tc:
        with tc.tile_pool(name="persist", bufs=1) as persist:
            ident = persist.tile([128, 128], f32)
            make_identity(nc, ident[:])
            ident025 = persist.tile([128, 128], f32)
            nc.vector.tensor_scalar_mul(ident025[:], ident[:], 0.25)
            identr = persist.tile([128, 128], f32r)
            nc.vector.tensor_copy(identr[:], ident[:])
            identb = persist.tile([128, 128], bf16)
            nc.vector.tensor_copy(identb[:], ident[:])
            sel12 = persist.tile([12, 1536], bf16)
            delta = persist.tile([B, S, QS], bf16)
            arbuf = persist.tile([B, ARF], f32)
            nc.vector.memset(arbuf[:, 1536:ARF], 0.0)
            post = persist.tile([B, EMOFF], bf16)
            F12 = persist.tile([B, 12], f32)
            E12 = persist.tile([B, 12], f32)
            D2s = persist.tile([B, 12], f32)
            DHs = persist.tile([B, 12], f32)
            DHh = persist.tile([B, 12], f32)
            ones_col = persist.tile([B, 1], f32)
            nc.vector.memset(ones_col[:], 1.0)
            blogt = persist.tile([B, 1], f32)
            nc.vector.memset(blogt[:], blog)

            with (
                tc.tile_pool(name="xload", bufs=3) as xload,
                tc.tile_pool(name="yload", bufs=3) as yload,
                tc.tile_pool(name="sbp", bufs=4) as sbp,
                tc.tile_pool(name="tpsum", bufs=4, space="PSUM") as tpsum,
                tc.tile_pool(name="bload", bufs=2) as bload,
                tc.tile_pool(name="scr", bufs=2) as scr,
                tc.tile_pool(name="epool", bufs=1) as epool,
                tc.tile_pool(name="gpsum", bufs=2, space="PSUM") as gpsum,
            ):
                # ---------- gram sweep for one pair ----------
                pend = []

                def sweep(p):
                    gp = gpsum.tile([128, 4, 128], f32, tag="gp", bufs=1)
                    for ci, (t0, w) in enumerate(CH):
                        xt_ = xload.tile([B, w, QS], f32, tag="x")
                        nc.sync.dma_start(out=xt_[:], in_=xs[p][:, t0:t0 + w, :])
                        yt_ = yload.tile([B, w, QS], f32, tag="y")
                        nc.sync.dma_start(out=yt_[:], in_=ys[p][:, t0:t0 + w, :])
                        xcb = xload.tile([B, w, QS], bf16, tag="xcb",
                                         name="xcb", bufs=2)
                        nc.scalar.copy(xcb[:], xt_[:])
                        ycb = yload.tile([B, w, QS], bf16, tag="ycb",
                                         name="ycb", bufs=2)
                        nc.scalar.copy(ycb[:], yt_[:])
                        ngr = (w + 1) // 2
                        pss = []
                        for g in range(ngr):
                            j0 = 2 * g
                            nt = min(2, w - j0)
                            ps = tpsum.tile([128, 4, 128], bf16, tag="tp",
                                            bufs=3, name="ps")
                            for u in range(nt):
                                nc.tensor.transpose(ps[:, 2 * u, :],
                                                    xcb[:, j0 + u, :],
                                                    identb[:])
                                nc.tensor.transpose(ps[:, 2 * u + 1, :],
                                                    ycb[:, j0 + u, :],
                                                    identb[:])
                            sb = sbp.tile([128, 4, 128], bf16, tag="sb",
                                          bufs=3, name="sb")
                            if g % 3 != 2:
                                nc.vector.tensor_copy(sb[:], ps[:])
                            else:
                                nc.scalar.copy(sb[:], ps[:])
                            pss.append((sb, j0, nt))
                            if len(pss) >= 3:
                                self_mm(p, gp, *pss.pop(0), t0)
                        while pss:
                            self_mm(p, gp, *pss.pop(0), t0)
                        if t0 < S:
                            s1 = min(t0 + w, S)
                            ns = s1 - t0
                            ms = scr.tile([B, 25, QS], bf16, tag="ms", bufs=2)
                            nc.gpsimd.tensor_mul(ms[:, 0:ns, :], xt_[:, 0:ns, :],
                                                 delta[:, t0:s1, :])
                            pend.append((ms, t0, s1, ns, p))
                    # defer pc reductions to sweep end so staging copies
                    # are never stuck behind them on the DVE queue
                    while pend:
                        ms, t0, s1, ns, pp = pend.pop(0)
                        nc.vector.reduce_sum(
                            out=arbuf[:, PCOFF[pp] + t0:PCOFF[pp] + s1],
                            in_=ms[:, 0:ns, :], axis=X)
                    nc.vector.tensor_copy(
                        arbuf[:, GOFF[p]:GOFF[p] + 512],
                        gp[:].rearrange("b k q -> b (k q)"))

                def self_mm(p, gp, sb, j0, nt, t0):
                    for u in range(nt):
                        kk = t0 + j0 + u
                        rhs = sb[:, 2 * u:2 * u + 2, :].rearrange(
                            "b u q -> b (u q)")
                        nc.tensor.matmul(
                            gp[:, 0:2, :].rearrange("b u q -> b (u q)"),
                            sb[:, 2 * u, :], rhs,
                            start=(kk == 0), stop=(kk == T - 1))
                        nc.tensor.matmul(
                            gp[:, 2:4, :].rearrange("b u q -> b (u q)"),
                            sb[:, 2 * u + 1, :], rhs,
                            start=(kk == 0), stop=(kk == T - 1))

                # ---------- fast dve ln: |err| <= 0.06 abs, plenty here ----
                def ln_dve(dst, src, n, tagp):
                    LN2 = 0.6931471805599453
                    ef = chp.tile([B, n], f32, tag=f"lef{tagp}")
                    nc.vector.tensor_copy(ef[:], src.bitcast(i32))
                    nc.vector.tensor_scalar(dst, ef[:], LN2 / (1 << 23),
                                            -126.957 * LN2, Alu.mult, Alu.add)

                # ---------------- emission: phase A ----------------
                sel12f = scr.tile([12, 1536], f32, tag="self", bufs=1)
                nc.sync.dma_start(out=sel12f[:], in_=sel_dram[:, :])
                nc.vector.tensor_copy(sel12[:], sel12f[:])
                for (t0, ns) in CHB:
                    s1 = t0 + ns
                    bct = bload.tile([B, 10, QS], f32, tag="bc")
                    nc.sync.dma_start(out=bct[:, 0:ns, :], in_=dbc[:, t0:s1, :])
                    bnt = bload.tile([B, 10, QS], f32, tag="bn")
                    nc.sync.dma_start(out=bnt[:, 0:ns, :], in_=dbn[:, t0:s1, :])
                    nc.vector.tensor_add(delta[:, t0:s1, :], bct[:, 0:ns, :],
                                         bnt[:, 0:ns, :])
                    dif = scr.tile([B, 10, QS], f32, tag="dif", bufs=1)
                    nc.vector.tensor_sub(dif[:, 0:ns, :], bct[:, 0:ns, :],
                                         bnt[:, 0:ns, :])
                    nc.vector.reduce_sum(out=arbuf[:, AOFF + t0:AOFF + s1],
                                         in_=dif[:, 0:ns, :], axis=X)

                sweep(0)
                sweep(1)
                sweep(2)

                # embeds (DMA tail)
                EH = EPAD * H // 2
                ecols = persist.tile([B, 4], f32)
                for ci, (ea, eb) in enumerate(((ehs, eht), (eds, edt))):
                    for hf in range(2):
                        e1 = epool.tile([B, EH], f32, tag="ea")
                        nc.sync.dma_start(
                            out=e1[:],
                            in_=ea[:].rearrange("b t h -> b (t h)")[
                                :, EH * hf:EH * (hf + 1)])
                        e2 = epool.tile([B, EH], f32, tag="eb")
                        nc.sync.dma_start(
                            out=e2[:],
                            in_=eb[:].rearrange("b t h -> b (t h)")[
                                :, EH * hf:EH * (hf + 1)])
                        ed = epool.tile([B, EH], f32, tag="ed")
                        nc.vector.tensor_sub(ed[:], e1[:], e2[:])
                        esq = epool.tile([B, EH], f32, tag="esq")
                        nc.scalar.activation(
                            esq[:], ed[:], Act.Square,
                            accum_out=ecols[:, 2 * ci + hf:2 * ci + hf + 1])
                emf = persist.tile([B, 1], f32)
                nc.vector.reduce_sum(out=emf[:], in_=ecols[:], axis=X)
                # bf16 ring additions would round the big embed sum, so give
                # each core its own hi/lo column pair (adding zeros is exact);
                # these ride the tiny AR2 so AR1 need not wait for embeds.
                emhi = persist.tile([B, 1], bf16)
                nc.vector.tensor_copy(emhi[:], emf[:])
                emhif = persist.tile([B, 1], f32)
                nc.vector.tensor_copy(emhif[:], emhi[:])
                emlo = persist.tile([B, 1], f32)
                nc.vector.tensor_sub(emlo[:], emf[:], emhif[:])
                emk = persist.tile([B, 8], f32)
                nc.sync.dma_start(out=emk[:], in_=emask[:, :])

                # ---------------- single AllReduce (bf16) ----------------
                arb16 = persist.tile([B, EMOFF], bf16)
                nc.vector.tensor_copy(arb16[:], arbuf[:, 0:EMOFF])
                nc.gpsimd.dma_start(out=ar_in[:, :], in_=arb16[:])
                nc.gpsimd.collective_compute(
                    "AllReduce",
                    mybir.AluOpType.add,
                    replica_groups=[list(range(NCORES))],
                    ins=[ar_in[:, :]],
                    outs=[ar_out[:, :]],
                )
                nc.gpsimd.dma_start(out=post[:, :], in_=ar_out[:, :])

            with (
                tc.tile_pool(name="chain", bufs=1) as chp,
                tc.tile_pool(name="scrB", bufs=2) as scr,
                tc.tile_pool(name="hpsum", bufs=1, space="PSUM") as hpsum,
                tc.tile_pool(name="spsum", bufs=1, space="PSUM") as spsum,
            ):
                # ------- phase B: per-core pair-sharded sinkhorn -------
                pmt = chp.tile([B, 4], f32, tag="pmt")
                nc.sync.dma_start(out=pmt[:], in_=pmask[:, :])
                myGa = chp.tile([B, 512], f32, tag="myGa")
                nc.vector.tensor_scalar(myGa[:], post[:, 0:512], pmt[:, 0:1],
                                        None, Alu.mult)
                myGb = chp.tile([B, 512], f32, tag="myGb")
                nc.vector.scalar_tensor_tensor(myGb[:], post[:, 512:1024],
                                               pmt[:, 1:2], myGa[:],
                                               Alu.mult, Alu.add)
                myGf = chp.tile([B, 512], f32, tag="myGf")
                nc.vector.scalar_tensor_tensor(myGf[:], post[:, 1024:1536],
                                               pmt[:, 2:3], myGb[:],
                                               Alu.mult, Alu.add)
                Gsb = chp.tile([B, 512], f32r, tag="gsb")
                nc.vector.tensor_copy(Gsb[:], myGf[:])

                dv6 = persist.tile([B, 2], f32)
                for col, blk in ((0, 0), (1, 3)):
                    dsc = scr.tile([B, 128], f32, tag="dsc")
                    nc.vector.tensor_mul(dsc[:],
                                         myGf[:, 128 * blk:128 * (blk + 1)],
                                         ident[:])
                    nc.vector.reduce_sum(out=dv6[:, col:col + 1], in_=dsc[:],
                                         axis=X)
                dxx, dyy = dv6[:, 0:1], dv6[:, 1:2]
                for col, src in ((0, dxx), (1, dxx), (2, dyy), (3, dyy)):
                    nc.vector.tensor_scalar_mul(D2s[:, 0:4][:, col:col + 1], src, 2.0)
                for col, src in ((0, dxx), (1, dyy), (2, dxx), (3, dyy)):
                    nc.vector.tensor_scalar_mul(DHs[:, 0:4][:, col:col + 1], src, -2.0)
                nc.vector.tensor_scalar_mul(DHh[:, 0:4], DHs[:, 0:4], 0.5)
                nc.vector.tensor_copy(F12[:, 0:4], DHs[:, 0:4])

                hb = hpsum.tile([128, 4, 128], f32, tag="hb")
                HTp = chp.tile([4, 128], f32, tag="htp")
                for ei, eps in enumerate(eps_list):
                    damp = 1.0 / (1.0 + eps / RHO)
                    c = GSCALE / eps
                    ftp = spsum.tile([4, 128], f32, tag="ftp", bufs=1)
                    nc.tensor.transpose(ftp[:], F12[:, 0:4], ident025[:])
                    HTb = chp.tile([4, 128], bf16, tag="ht")
                    if ei == 0:
                        nc.vector.tensor_copy(HTb[:], ftp[:])
                        nc.vector.tensor_copy(HTp[:], ftp[:])
                        nc.tensor.matmul(
                            hb[:].rearrange("b k q -> b (k q)"),
                            identr[:], Gsb[:],
                            start=True, stop=False, skip_group_check=True)
                    else:
                        dhtf = chp.tile([4, 128], f32, tag="dhtf")
                        nc.vector.tensor_sub(dhtf[:], ftp[:], HTp[:])
                        nc.vector.tensor_copy(HTb[:], dhtf[:])
                        nc.vector.tensor_copy(HTp[:], ftp[:])
                    last = (ei == len(eps_list) - 1)
                    for m in range(4):
                        nc.tensor.matmul(hb[:, m, :],
                                         sel12[0:4, 128 * m:128 * (m + 1)],
                                         HTb[:], start=False, stop=last,
                                         skip_group_check=True)
                    mv = chp.tile([B, 4], f32, tag="mv")
                    nc.vector.reduce_max(out=mv[:], in_=hb[:], axis=X)
                    mb = mv[:].unsqueeze(2).broadcast_to((B, 4, 128))
                    scrt = chp.tile([B, 4, 128], bf16, tag="scrt")
                    nc.vector.tensor_tensor(scrt[:], hb[:], mb, Alu.subtract)
                    scre = chp.tile([B, 4, 128], bf16, tag="scre")
                    nc.scalar.activation(scre[:], scrt[:], Act.Exp,
                                         bias=blogt[:, 0:1], scale=float(c))
                    sv = chp.tile([B, 4], f32, tag="sv")
                    nc.vector.reduce_sum(out=sv[:], in_=scre[:], axis=X)
                    lg = chp.tile([B, 4], f32, tag="lg")
                    ln_dve(lg[:], sv[:], 4, "a")
                    z = chp.tile([B, 4], f32, tag="z")
                    nc.vector.scalar_tensor_tensor(z[:], lg[:], float(eps / 4.0),
                                                   mv[:], Alu.mult, Alu.add)
                    dD2 = chp.tile([B, 4], f32, tag="dd2")
                    nc.vector.tensor_scalar_mul(dD2[:], D2s[:, 0:4], float(damp))
                    cand = chp.tile([B, 4], f32, tag="cand")
                    nc.vector.scalar_tensor_tensor(cand[:], z[:],
                                                   float(-4.0 * damp), dD2[:],
                                                   Alu.mult, Alu.add)
                    nc.vector.tensor_add(F12[:, 1:2], cand[:, 2:3],
                                         DHs[:, 1:2])
                    nc.vector.tensor_add(F12[:, 2:3], cand[:, 1:2],
                                         DHs[:, 2:3])
                    for k in (0, 3):
                        t1 = chp.tile([B, 1], f32, tag="t1")
                        nc.vector.scalar_tensor_tensor(
                            t1[:], cand[:, k:k + 1], 0.5, DHh[:, k:k + 1],
                            Alu.mult, Alu.add)
                        nc.vector.scalar_tensor_tensor(
                            F12[:, k:k + 1], F12[:, k:k + 1], 0.5, t1[:],
                            Alu.mult, Alu.add)

                Ft = chp.tile([B, 4], f32, tag="ftt")
                nc.vector.tensor_sub(Ft[:], F12[:, 0:4], DHs[:, 0:4])
                nc.scalar.activation(E12[:, 0:4], Ft[:], Act.Exp,
                                     scale=float(-1.0 / RHO))
                # place E4 into my pair slot, AllReduce the 12 columns
                e12b = chp.tile([B, 32], bf16, tag="e12b")
                nc.vector.memset(e12b[:], 0.0)
                for p in range(3):
                    nc.vector.tensor_scalar(e12b[:, 4 * p:4 * p + 4],
                                            E12[:, 0:4], pmt[:, p:p + 1],
                                            None, Alu.mult)
                nc.vector.tensor_scalar(e12b[:, 16:24], emk[:],
                                        emhif[:, 0:1], None, Alu.mult)
                nc.vector.tensor_scalar(e12b[:, 24:32], emk[:],
                                        emlo[:, 0:1], None, Alu.mult)
                nc.gpsimd.dma_start(out=ar2_in[:, :], in_=e12b[:])
                nc.gpsimd.collective_compute(
                    "AllReduce",
                    mybir.AluOpType.add,
                    replica_groups=[list(range(NCORES))],
                    ins=[ar2_in[:, :]],
                    outs=[ar2_out[:, :]],
                )
                post2 = chp.tile([B, 32], bf16, tag="post2")
                nc.gpsimd.dma_start(out=post2[:], in_=ar2_out[:, :])
                E12f = chp.tile([B, 12], f32, tag="e12f")
                nc.vector.tensor_copy(E12f[:], post2[:, 0:12])

                # ---------------- CE ----------------
                idxf = persist.tile([B, 64], f32)
                nc.sync.dma_start(out=idxf[:], in_=idx_dram[:, :])
                pcb = post[:, PCOFF[0]:PCOFF[0] + 64]
                pos = chp.tile([B, 64], f32, tag="pos")
                nc.vector.tensor_scalar(pos[:], pcb, 0.0, None, Alu.is_gt)
                ip1 = chp.tile([B, 64], f32, tag="ip1")
                nc.vector.scalar_tensor_tensor(ip1[:], idxf[:], 1.0, pos[:],
                                               Alu.add, Alu.mult)
                Lp = chp.tile([B, 1], f32, tag="Lp")
                nc.vector.reduce_max(out=Lp[:], in_=ip1[:], axis=X)
                eq0 = chp.tile([B, 1], f32, tag="eq0")
                nc.vector.tensor_scalar(eq0[:], Lp[:], 0.0, None, Alu.is_equal)
                Lv = chp.tile([B, 1], f32, tag="Lv")
                nc.vector.scalar_tensor_tensor(Lv[:], eq0[:], float(S), Lp[:],
                                               Alu.mult, Alu.add)
                dl = chp.tile([B, 64], f32, tag="dl")
                nc.vector.tensor_scalar(dl[:], idxf[:], Lv[:, 0:1], None,
                                        Alu.subtract)
                mask = chp.tile([B, 64], f32, tag="mask")
                nc.vector.tensor_scalar(mask[:], dl[:], 0.0, None, Alu.is_lt)
                negf = chp.tile([B, 64], f32, tag="negf")
                nc.vector.tensor_scalar(negf[:], mask[:], 1.0, 1e9,
                                        Alu.subtract, Alu.mult)
                MAGIC = 12582912.0
                tv = chp.tile([B, 64], f32, tag="tv")
                nc.vector.tensor_scalar(tv[:], post[:, AOFF:AOFF + 64], 0.5,
                                        1024.25, Alu.mult, Alu.add)
                tm = chp.tile([B, 64], f32, tag="tm")
                nc.vector.tensor_scalar(tm[:], tv[:], MAGIC, MAGIC,
                                        Alu.add, Alu.subtract)
                av = chp.tile([B, 64], f32, tag="av")
                nc.vector.tensor_scalar(av[:], tm[:], 1024.0, None, Alu.subtract)
                amask = chp.tile([B, 64], f32, tag="amask")
                nc.vector.tensor_tensor(amask[:], av[:], mask[:], Alu.mult)
                pc3 = post[:, PCOFF[0]:PCOFF[0] + 192].rearrange(
                    "b (s q) -> b s q", q=64)
                mce = chp.tile([B, 3, 64], f32, tag="mce")
                mask3 = mask[:].unsqueeze(1).broadcast_to((B, 3, 64))
                negf3 = negf[:].unsqueeze(1).broadcast_to((B, 3, 64))
                amask3 = amask[:].unsqueeze(1).broadcast_to((B, 3, 64))
                t2_ = chp.tile([B, 3, 64], f32, tag="tt")
                nc.vector.scalar_tensor_tensor(t2_[:], pc3, 2.0, mask3, Alu.mult,
                                               Alu.mult)
                nc.vector.tensor_tensor(mce[:], t2_[:], negf3, Alu.add)
                mx3 = chp.tile([B, 3], f32, tag="mx3")
                nc.vector.reduce_max(out=mx3[:], in_=mce[:], axis=X)
                mb3 = mx3[:].unsqueeze(2).broadcast_to((B, 3, 64))
                dd = chp.tile([B, 3, 64], f32, tag="dd")
                nc.vector.tensor_tensor(dd[:], mce[:], mb3, Alu.subtract)
                ee = chp.tile([B, 3, 64], f32, tag="ee")
                nc.scalar.activation(ee[:], dd[:], Act.Exp)
                ss3 = chp.tile([B, 3], f32, tag="ss3")
                nc.vector.reduce_sum(out=ss3[:], in_=ee[:], axis=X)
                lg3 = chp.tile([B, 3], f32, tag="lg3")
                ln_dve(lg3[:], ss3[:], 3, "b")
                lse3 = chp.tile([B, 3], f32, tag="lse3")
                nc.vector.tensor_add(lse3[:], mx3[:], lg3[:])
                lb3 = lse3[:].unsqueeze(2).broadcast_to((B, 3, 64))
                d1 = chp.tile([B, 3, 64], f32, tag="dd")
                nc.vector.tensor_tensor(d1[:], mce[:], lb3, Alu.subtract)
                d2_ = chp.tile([B, 3, 64], f32, tag="tt")
                nc.vector.tensor_tensor(d2_[:], d1[:], amask3, Alu.mult)
                rowsum = chp.tile([B, 1], f32, tag="rs")
                nc.vector.reduce_sum(out=rowsum[:],
                                     in_=d2_[:].rearrange("b s q -> b (s q)"),
                                     axis=X)

                # ---------------- final combine ----------------
                cs_ps = spsum.tile([12, 1], f32, tag="ftp", bufs=1)
                nc.tensor.matmul(cs_ps[:], E12f[:], ones_col[:], start=True,
                                 stop=True)
                cs = chp.tile([12, 1], f32, tag="css")
                nc.vector.tensor_copy(cs[:], cs_ps[:])
                coeff = persist.tile([12, 1], f32)
                nc.sync.dma_start(out=coeff[:], in_=coeff_dram[:, :])
                csup = persist.tile([B, 1], f32)
                nc.vector.memset(csup[:], float(-LOSS_WEIGHT * SUP_W))
                cemb = persist.tile([B, 1], f32)
                nc.vector.memset(cemb[:], float(LOSS_WEIGHT * EMBED_W * 0.5))
                tot_ps = spsum.tile([1, 1], f32, tag="ftp", bufs=1)
                nc.tensor.matmul(tot_ps[:], rowsum[:], csup[:], start=True,
                                 stop=False, skip_group_check=True)
                emsum = chp.tile([B, 1], f32, tag="emsum")
                nc.vector.reduce_sum(out=emsum[:], in_=post2[:, 16:32], axis=X)
                nc.tensor.matmul(tot_ps[:], emsum[:], cemb[:],
                                 start=False, stop=False, skip_group_check=True)
                nc.tensor.matmul(tot_ps[:], cs[:], coeff[:], start=False,
                                 stop=True, skip_group_check=True)
                outt = chp.tile([1, 1], f32, tag="outt")
                nc.vector.tensor_copy(outt[:], tot_ps[:])
                nc.sync.dma_start(out=out_ext[:, :], in_=outt[:])

    nc.compile()
    return nc


_NC = None
LAST_RESULTS = None


def _shard_inputs(logit_c, logit_t, logit_ensemble, logit_teacher_c,
                  logit_teacher_t, logit_teacher_ensemble, out_h_student,
                  out_h_teacher, out_d_student, out_d_teacher, batch):
    asf = lambda a: np.ascontiguousarray(a, dtype=np.float32)
    students = [logit_c, logit_t, logit_ensemble]
    teachers = [logit_teacher_c, logit_teacher_t, logit_teacher_ensemble]
    embeds = dict(ehs=out_h_student, eht=out_h_teacher,
                  eds=out_d_student, edt=out_d_teacher)
    in_maps = []
    for c in range(NCORES):
        q0 = QS * c
        m = {}
        for nm, arr in zip(("xc", "xt", "xe"), students):
            m[nm] = asf(arr[:, :, q0:q0 + QS])
        for nm, arr in zip(("yc", "yt", "ye"), teachers):
            m[nm] = asf(arr[:, :, q0:q0 + QS])
        m["dbc"] = asf(batch[:, 1:1 + S, q0:q0 + QS])
        m["dbn"] = asf(batch[:, 1:1 + S, Q + q0:Q + q0 + QS])
        mk = np.zeros((B, 8), np.float32)
        mk[:, c] = 1.0
        m["emask"] = mk
        pm = np.zeros((B, 4), np.float32)
        if c < 3:
            pm[:, c] = 1.0
        m["pmask"] = pm
        t0, w = EOFF[c], ESPLIT[c]
        for nm, arr in embeds.items():
            sl = np.zeros((B, EPAD, H), np.float32)
            sl[:, :w, :] = np.asarray(arr[:, t0:t0 + w, :], dtype=np.float32)
            m[nm] = sl
        in_maps.append(m)
    return in_maps


def kernel(**inputs):
    global _NC, LAST_RESULTS
    from concourse.bass_utils import run_bass_kernel_spmd
    if _NC is None:
        _NC = build_bass()
    in_maps = _shard_inputs(**inputs)
    trace = bool(int(os.environ.get("KERNEL_TRACE", "0")))
    res = run_bass_kernel_spmd(_NC, in_maps, list(range(NCORES)), trace=trace)
    LAST_RESULTS = res
    return np.asarray(res.results[0]["out"], dtype=np.float32).reshape(1)
